# revision 3
# baseline (speedup 1.0000x reference)
"""Block-diagonal linear layer on 8 trn2 NeuronCores.

Reference op:  out = x @ tanh(W * mask).T
  x    [8192, 4096] f32
  W    [4096, 4096] f32, random inside 8 diagonal 512x512 blocks, 0 outside
  mask [4096, 4096] bool, True exactly on the 8 diagonal 512x512 blocks

tanh(0) == 0, so eff = tanh(W*mask) is block-diagonal: out[:, blk_k] depends
only on x[:, blk_k] and W[blk_k, blk_k].  Sharding: block k -> core k
(expert-style), zero inter-core communication.

Mixed precision: 18 of the 64 (tile, o) output groups run as fp8 e4m3
DoubleRow matmuls (batch tiles 0-3 fully + tile 4's o-groups 0,1); the rest
is f16 at the trn2 16-bit roofline (1 col/cycle).  Host-simulated (bit-true
to 5 digits vs HW) total rel err = 1.987e-2 < the 2e-2 gate.  Full fp8 would
be 3.75e-2; e3m4 / int8 / DoublePixel paths were probed and are rejected by
the walrus verifier or the cost model, so e4m3-DR at 2x is the frontier.

Schedule (fp8-first ramp, from trace analysis of the 67.0us baseline):
  head   framework preamble ends ~6.1us; pool barrier ~7.0us.  The fp8
         critical set [x8 tile0 | w8] is ONE 512KB DMA on the sync ring
         (the faster-starting HWDGE ring, data from ~8.7us) so the first
         real DR matmul lands ~2.1us earlier than the baseline's 1MB
         f16-critical transfer.  x8 tiles 1-3 follow as separate 256KB
         DMAs (sems fire incrementally, matching the 432ns/group burn
         rate), then x8-tile4, eff (f16), then f16 x tiles on both rings.
         Warmup matmuls keep the PE busy from ~8.0us so the HAM clock
         gate (3.4us busy window) opens before/at the real stream.
  body   fp8 phase: 18 DR groups at 432ns/group, drains alternating
         vector/scalar (each ~680ns < the 864ns per-engine period).
         f16 phase: 46 groups at 864ns, vector drains.  Quads 0-1 store
         512KB per o-group after the quad; quads 2-3 store 128KB per
         (h,o) group, alternating rings, so output DMA is spread evenly.
  tail   the final group drains as two 256-col halves on both rings so
         the last 64KB store issues right after the last matmul (the
         ~1.4us HBM write-completion latency is the tail floor).  The
         ~7.3us of per-semaphore teardown after the final barrier is
         injected by the runtime outside the NEFF; every kernel pays it.
"""

from contextlib import ExitStack

import numpy as np

BLOCK = 512
NBLOCKS = 8
BATCH = 8192
N = BLOCK * NBLOCKS

KI = BLOCK // 128   # 4 contraction chunks of 128 (SBUF partition dim)
OT = BLOCK // 128   # 4 output-row tiles of 128
BT = 512            # batch tile (one PSUM bank of f32)
NB = BATCH // BT    # 16 batch tiles
QUAD = 2048         # batch columns per staging tile
NT8 = 4             # batch tiles fully in fp8 (tiles 0-3)
NE8 = 2             # extra fp8 o-groups in tile 4 (o=0,1)
NF16 = NB - NT8     # f16-loaded tiles (4..15)

SXQ = 32.0          # fp8 scale for x   (max|x|*32  ~ 174 < 240)
SWQ = 1024.0        # fp8 scale for eff (max|eff|*1024 ~ 217 < 240)
DQ = 1.0 / (SXQ * SWQ)

NWARM = 14          # 256-col warmup matmuls (~224ns each cold)

_CACHED = {}


def _build_program():
    import concourse.bacc as bacc
    import concourse.mybir as mybir
    import concourse.tile as tile

    f16 = mybir.dt.float16
    f32 = mybir.dt.float32
    f8 = mybir.dt.float8e4
    DR = mybir.MatmulPerfMode.DoubleRow

    nc = bacc.Bacc(
        "TRN2",
        target_bir_lowering=False,
        debug=False,
        enable_asserts=False,
        num_devices=NBLOCKS,
    )

    # c8 packs x8-tile0 and the full fp8 eff^T into ONE 512KB / 128-descriptor
    # transfer: cold-start DMA is descriptor+byte-rate bound, so the critical
    # set must be one small transfer.  c8[p,0,g,i,b] = x8 tile0; c8[p,1,g,i,
    # 128o+j] = w8[o,g,i,j] (o,j packed in the last dim so both views slice
    # out of one tile).
    c8d = nc.dram_tensor("c8", [128, 2, 2, 2, BT], f8, kind="ExternalInput").ap()
    x8rd = nc.dram_tensor("x8r", [128, 3, 2, 2, BT], f8, kind="ExternalInput").ap()
    x8ed = nc.dram_tensor("x8e", [128, 2, 2, BT], f8, kind="ExternalInput").ap()
    efftd = nc.dram_tensor("efft", [128, KI, BLOCK], f16, kind="ExternalInput").ap()
    xfd = nc.dram_tensor("xf", [128, NF16, KI, BT], f16, kind="ExternalInput").ap()
    ot = nc.dram_tensor("ot", [BLOCK, BATCH], f16, kind="ExternalOutput").ap()

    with tile.TileContext(nc) as tc, ExitStack() as ctx:
        wpool = ctx.enter_context(tc.tile_pool(name="w", bufs=1))
        opool = ctx.enter_context(tc.tile_pool(name="o", bufs=3))
        pspool = ctx.enter_context(tc.tile_pool(name="ps", bufs=2, space="PSUM"))

        # PE warmup: keeps the PE busy from ~8.0us so the HAM clock-gate's
        # 3.4us busy window completes by the time the real stream runs.
        xwarm = wpool.tile([128, 256], f16, tag="warm", name="xwarm")
        nc.vector.memset(xwarm[:], 0.0)
        pw = pspool.tile([128, BT], f32, tag="pb0", name="warm")
        for r in range(NWARM):
            nc.tensor.matmul(
                pw[:, 0:256], xwarm[:, 0:128], xwarm[:, 0:256], start=True, stop=True
            )

        c8 = wpool.tile([128, 2, 2, 2, BT], f8, tag="c8", name="c8")
        x8r = wpool.tile([128, 3, 2, 2, BT], f8, tag="x8r", name="x8r")
        x8e = wpool.tile([128, 2, 2, BT], f8, tag="x8e", name="x8e")
        efft = wpool.tile([128, KI, BLOCK], f16, tag="efft", name="efft")
        xf = wpool.tile([128, NF16, KI, BT], f16, tag="xf", name="xf")

        # sync ring: the fp8 ramp, then eff, then late f16 tiles.
        nc.sync.dma_start(c8[:], c8d[:])
        nc.sync.dma_start(x8r[:, 0], x8rd[:, 0])
        nc.sync.dma_start(x8r[:, 1], x8rd[:, 1])
        # scalar ring starts slow (~3us later): early f16 tiles + ACT preload.
        nc.scalar.dma_start(xf[:, 0], xfd[:, 0])
        # tiny dummy activation: hoists the scalar ACT_TABLE_LOAD into the
        # startup DMA-wait dead time, so the fp8 scalar drains don't pay it.
        nc.scalar.activation(
            xwarm[:, 0:1], xwarm[:, 0:1], mybir.ActivationFunctionType.Copy
        )
        nc.scalar.dma_start(xf[:, 1], xfd[:, 1])
        nc.scalar.dma_start(xf[:, 2:5], xfd[:, 2:5])
        nc.sync.dma_start(x8r[:, 2], x8rd[:, 2])
        nc.sync.dma_start(x8e[:], x8ed[:])
        nc.sync.dma_start(efft[:], efftd[:])
        nc.sync.dma_start(xf[:, 5:8], xfd[:, 5:8])
        nc.sync.dma_start(xf[:, 8:12], xfd[:, 8:12])

        def w8v(o, g):  # fp8 eff^T view [128 (i=2 interleave), 128 out] for DR
            return c8[:, 1, g, :, 128 * o : 128 * (o + 1)]

        def x8v(t, g):  # fp8 x view [128, (2), 512] for DR
            if t == 0:
                return c8[:, 0, g, :, :]
            if t < 4:
                return x8r[:, t - 1, g, :, :]
            return x8e[:, g, :, :]

        def dr_group(t, o, dest, gi):
            """One fp8 DoubleRow accumulation group -> dest (f16 staging)."""
            p8 = pspool.tile([128, BT], f32, tag=f"pb{o}", name=f"p8_{t}_{o}")
            for g in range(2):
                nc.tensor.matmul(
                    p8[:],
                    w8v(o, g),
                    x8v(t, g),
                    start=(g == 0),
                    stop=(g == 1),
                    perf_mode=DR,
                )
            if gi % 2 == 0:
                nc.vector.tensor_scalar_mul(dest, p8[:], DQ)
            else:
                nc.scalar.activation(
                    dest, p8[:], mybir.ActivationFunctionType.Copy, scale=DQ
                )

        def f16_group(t, o, dest, drain):
            """One f16 accumulation group (K=512) -> dest (f16 staging)."""
            ps = pspool.tile([128, BT], f32, tag=f"pb{o}", name=f"pf_{t}_{o}")
            for i in range(KI):
                nc.tensor.matmul(
                    ps[:],
                    efft[:, i, 128 * o : 128 * (o + 1)],
                    xf[:, t - NT8, i, :],
                    start=(i == 0),
                    stop=(i == KI - 1),
                )
            drain(dest, ps)

        vcopy = lambda d, ps: nc.vector.tensor_copy(d, ps[:])

        # ---- Phase A: fp8 quad 0 (tiles 0-3, all o) ----
        stg0 = [
            opool.tile([128, QUAD], f16, tag=f"so{o}", name=f"st0_{o}")
            for o in range(OT)
        ]
        gi = 0
        for t in range(NT8):
            for o in range(OT):
                dr_group(t, o, stg0[o][:, BT * t : BT * (t + 1)], gi)
                gi += 1
        # ---- tile 4's fp8 o-groups (into quad-1 staging) ----
        stg1 = [
            opool.tile([128, QUAD], f16, tag=f"so{o}", name=f"st1_{o}")
            for o in range(OT)
        ]
        for o in range(NE8):
            dr_group(4, o, stg1[o][:, 0:BT], gi)
            gi += 1
        # quad 0 stores (512KB per o, alternating rings)
        for o in range(OT):
            eng = nc.sync if o % 2 == 0 else nc.scalar
            eng.dma_start(ot[128 * o : 128 * (o + 1), 0:QUAD], stg0[o][:])

        # ---- Phase B: f16 quad 1 (tile 4 o2-3, tiles 5-7 all o) ----
        for o in range(NE8, OT):
            f16_group(4, o, stg1[o][:, 0:BT], vcopy)
        for t in range(5, 8):
            for o in range(OT):
                f16_group(t, o, stg1[o][:, BT * (t - 4) : BT * (t - 3)], vcopy)
        for o in range(OT):
            eng = nc.scalar if o % 2 == 0 else nc.sync
            eng.dma_start(ot[128 * o : 128 * (o + 1), QUAD : 2 * QUAD], stg1[o][:])

        # ---- Phases C/D: f16 quads 2-3, 128KB store per (h,o) group ----
        for q in (2, 3):
            stgs = [
                opool.tile([128, QUAD], f16, tag=f"so{o}", name=f"st{q}_{o}")
                for o in range(OT)
            ]
            for h in range(4):
                t = 4 * q + h
                for o in range(OT):
                    col0 = QUAD * q + BT * h
                    last = q == 3 and h == 3 and o == OT - 1
                    if not last:
                        f16_group(t, o, stgs[o][:, BT * h : BT * (h + 1)], vcopy)
                        eng = nc.sync if (h * OT + o) % 2 == 0 else nc.scalar
                        eng.dma_start(
                            ot[128 * o : 128 * (o + 1), col0 : col0 + BT],
                            stgs[o][:, BT * h : BT * (h + 1)],
                        )
                    else:
                        # final group: two 256-col half-groups so the first
                        # half's drain+store starts one half early and the
                        # last 64KB store issues right after the last matmul
                        ps = pspool.tile([128, BT], f32, tag=f"pb{o}", name="plast")
                        for half, eng in enumerate((nc.sync, nc.scalar)):
                            psl = slice(256 * half, 256 * (half + 1))
                            for i in range(KI):
                                nc.tensor.matmul(
                                    ps[:, psl],
                                    efft[:, i, 128 * o : 128 * (o + 1)],
                                    xf[:, t - NT8, i, 256 * half : 256 * (half + 1)],
                                    start=(i == 0),
                                    stop=(i == KI - 1),
                                )
                            sl = slice(BT * h + 256 * half, BT * h + 256 * (half + 1))
                            if half == 0:
                                nc.vector.tensor_copy(stgs[o][:, sl], ps[:, psl])
                            else:
                                nc.scalar.activation(
                                    stgs[o][:, sl],
                                    ps[:, psl],
                                    mybir.ActivationFunctionType.Copy,
                                )
                            eng.dma_start(
                                ot[
                                    128 * o : 128 * (o + 1),
                                    col0 + 256 * half : col0 + 256 * (half + 1),
                                ],
                                stgs[o][:, sl],
                            )

    nc.compile()
    return nc


def get_program():
    if "nc" not in _CACHED:
        _CACHED["nc"] = _build_program()
    return _CACHED["nc"]


def make_in_maps(x: np.ndarray, W: np.ndarray):
    import ml_dtypes

    e4 = ml_dtypes.float8_e4m3
    x = np.asarray(x, dtype=np.float32)
    W = np.asarray(W, dtype=np.float32)
    in_maps = []
    for k in range(NBLOCKS):
        sl = slice(BLOCK * k, BLOCK * (k + 1))
        xb = x[:, sl]  # [8192, 512] f32
        E16 = np.tanh(W[sl, sl]).astype(np.float16)  # [512 o, 512 i]
        Ef = np.tanh(W[sl, sl].astype(np.float64)).astype(np.float32)

        # x8[t][p, g, i, b] = q4(xb[512t+b, 128(2g+i)+p] * SXQ), tiles 0-4
        x8 = np.clip(
            xb[: BT * (NT8 + 1)].reshape(NT8 + 1, BT, 2, 2, 128).transpose(0, 4, 2, 3, 1)
            * SXQ,
            -240,
            240,
        ).astype(e4)  # [5, 128, 2, 2, 512]
        # w8[p, g, i, 128o+j] = q4(Ef[128o+j, 128(2g+i)+p] * SWQ)
        w8 = np.clip(
            Ef.reshape(OT, 128, 2, 2, 128).transpose(4, 2, 3, 0, 1) * SWQ, -240, 240
        ).astype(e4)  # [128, 2, 2, 4, 128] with (o,j) last
        c8 = np.ascontiguousarray(
            np.stack([x8[0], w8.reshape(128, 2, 2, BLOCK)], axis=1)
        )  # [128, 2, 2, 2, 512]
        x8r = np.ascontiguousarray(x8[1:4].transpose(1, 0, 2, 3, 4))
        x8e = np.ascontiguousarray(x8[4])
        # efft[p, c, o] = E16[o, 128c+p]
        efft = np.ascontiguousarray(E16.reshape(BLOCK, KI, 128).transpose(2, 1, 0))
        # xf[p, u, c, b] = x16[512(4+u)+b, 128c+p]
        xf = np.ascontiguousarray(
            xb[BT * NT8 :]
            .astype(np.float16)
            .reshape(NF16, BT, KI, 128)
            .transpose(3, 0, 2, 1)
        )
        in_maps.append(
            {"c8": c8, "x8r": x8r, "x8e": x8e, "efft": efft, "xf": xf}
        )
    return in_maps


def assemble_output(results) -> np.ndarray:
    out = np.empty((BATCH, N), np.float32)
    for k in range(NBLOCKS):
        out[:, BLOCK * k : BLOCK * (k + 1)] = results[k]["ot"].T.astype(np.float32)
    return out


def kernel(x: np.ndarray, W: np.ndarray, mask: np.ndarray) -> np.ndarray:
    # mask is exactly the block-diagonal pattern (all-True inside each
    # diagonal 512 block); W is already zero off-block, so tanh(W*mask)
    # restricted to block k is tanh(W[blk_k, blk_k]).
    from concourse.bass_utils import run_bass_kernel_spmd

    nc = get_program()
    in_maps = make_in_maps(x, W)
    res = run_bass_kernel_spmd(nc, in_maps, list(range(NBLOCKS)))
    return assemble_output(res.results)
